# revision 23
# baseline (speedup 1.0000x reference)
"""Bass/Trainium2 kernel for nn_Decoder (LSTM decoder + log_softmax over vocab).

Sharding: data-parallel over batch B=128 across 8 cores (16 rows each).
Per core: embedding gather -> h0/c0 projection -> precompute x-gates+bias
(spilled to DRAM) -> 32-step LSTM (recurrent matmuls only) -> output
projection streaming W_out^T in bf16 once; pass1 writes z+b tiles (fp16)
to DRAM and accumulates exp row-sums (fused on scalar engine); pass2
reloads z+b and writes z + b - logsumexp. No max-subtraction is needed:
|logits| <= ||h||*max||W_row|| + |b| < 30, far from fp32 exp overflow.
"""

import os
import sys

for _p in ("/opt/trn_rl_repo", "/root/.axon_site/_ro/trn_rl_repo"):
    if os.path.isdir(_p) and _p not in sys.path:
        sys.path.insert(0, _p)

import numpy as np
import ml_dtypes

import concourse.bass as bass
import concourse.tile as tile
from concourse import bacc
from concourse import mybir
from concourse.bass_utils import run_bass_kernel_spmd
from concourse.masks import make_identity

T, B, V, E, H, DW = 32, 128, 32000, 256, 512, 300
G = 4 * H            # 2048 gate width
NCORES = 8
BP = B // NCORES     # 16 batch rows per core
R = T * BP           # 512 local rows (t-major, then b)
NM = R // 128        # 4 m-tiles of 128 rows
NV = 500             # vocab tile width (<=512 fp32 PSUM bank)
NN = V // NV         # 64 vocab tiles

F32 = mybir.dt.float32
F16 = mybir.dt.float16
BF16 = mybir.dt.bfloat16
I32 = mybir.dt.int32
AF = mybir.ActivationFunctionType
ALU = mybir.AluOpType


def build_nc(skip_lstm=False, skip_b1=False, skip_b2=False):
    nc = bacc.Bacc()

    # ---- kernel I/O ----
    idx = nc.dram_tensor("idx", [128, NM], I32, kind="ExternalInput")
    emb_tab = nc.dram_tensor("emb_tab", [V, E], F32, kind="ExternalInput")
    embT = nc.dram_tensor("embT", [DW, BP], F32, kind="ExternalInput")
    WhT = nc.dram_tensor("WhT", [DW, H], F32, kind="ExternalInput")
    WcT = nc.dram_tensor("WcT", [DW, H], F32, kind="ExternalInput")
    bh = nc.dram_tensor("bh", [1, H], F32, kind="ExternalInput")
    bc = nc.dram_tensor("bc", [1, H], F32, kind="ExternalInput")
    WihT = nc.dram_tensor("WihT", [E, G], F32, kind="ExternalInput")
    WhhT = nc.dram_tensor("WhhT", [H, G], F32, kind="ExternalInput")
    ball = nc.dram_tensor("ball", [1, G], F32, kind="ExternalInput")
    WoT = nc.dram_tensor("WoT", [H, V], BF16, kind="ExternalInput")
    bo = nc.dram_tensor("bo", [1, V], BF16, kind="ExternalInput")

    out_lp = nc.dram_tensor("out_lp", [T, BP, V], F32, kind="ExternalOutput")
    out_h = nc.dram_tensor("out_h", [BP, H], F32, kind="ExternalOutput")
    out_c = nc.dram_tensor("out_c", [BP, H], F32, kind="ExternalOutput")

    # internal DRAM scratch
    xgb_dram = nc.dram_tensor("xgb_scratch", [R, G], F16)
    zb_dram = nc.dram_tensor("zb_scratch", [NM, NN // 2, 128, 2 * NV], F16)

    with tile.TileContext(nc) as tc:
        with (
            tc.tile_pool(name="const", bufs=1) as cpool,
            tc.tile_pool(name="gath", bufs=2) as gpool,
            tc.tile_pool(name="elem", bufs=2) as epool,
            tc.tile_pool(name="state", bufs=2) as spool,
            tc.tile_pool(name="wstr", bufs=3) as wpool,
            tc.tile_pool(name="evac", bufs=4) as vpool,
            tc.tile_pool(name="psum", bufs=1, space=bass.MemorySpace.PSUM) as ppool,
        ):
            # ---------------- constants into SBUF ----------------
            ident = cpool.tile([128, 128], F32)
            make_identity(nc, ident)
            ones = cpool.tile([1, 128], F32)
            nc.vector.memset(ones, 1.0)

            wih_sb = cpool.tile([128, E // 128, G], F32)
            nc.sync.dma_start(out=wih_sb, in_=WihT[:].rearrange("(c p) g -> p c g", p=128))
            whh_sb = cpool.tile([128, H // 128, G], F32)
            nc.sync.dma_start(out=whh_sb, in_=WhhT[:].rearrange("(c p) g -> p c g", p=128))
            ball_sb = cpool.tile([1, G], F32)
            nc.sync.dma_start(out=ball_sb, in_=ball[:])
            bh_sb = cpool.tile([1, H], F32)
            nc.sync.dma_start(out=bh_sb, in_=bh[:])
            bc_sb = cpool.tile([1, H], F32)
            nc.sync.dma_start(out=bc_sb, in_=bc[:])

            # emb^T and W_h^T / W_c^T come in DW=300 rows -> chunks 128,128,44
            KCH = [(0, 128), (128, 128), (256, DW - 256)]
            embT_sb = cpool.tile([128, 3, BP], F32)
            whT_sb = cpool.tile([128, 3, H], F32)
            wcT_sb = cpool.tile([128, 3, H], F32)
            for c, (k0, kn) in enumerate(KCH):
                nc.sync.dma_start(out=embT_sb[:kn, c, :], in_=embT[k0:k0 + kn, :])
                nc.sync.dma_start(out=whT_sb[:kn, c, :], in_=WhT[k0:k0 + kn, :])
                nc.sync.dma_start(out=wcT_sb[:kn, c, :], in_=WcT[k0:k0 + kn, :])

            # gather indices: [128, NM] col i holds rows i*128..i*128+127
            idx_sb = cpool.tile([128, NM], I32)
            nc.sync.dma_start(out=idx_sb, in_=idx[:])

            # ---------------- phase A: embedding gather -> e_seqT ----------------
            eT = cpool.tile([128, E // 128, R], F32)  # [E(2x128), rows]
            for i in range(NM):
                etile = gpool.tile([128, E], F32)
                nc.gpsimd.indirect_dma_start(
                    out=etile,
                    out_offset=None,
                    in_=emb_tab[:],
                    in_offset=bass.IndirectOffsetOnAxis(ap=idx_sb[:, i:i + 1], axis=0),
                )
                pst = ppool.tile([128, 256], F32, tag="tp", bufs=2)
                for c in range(E // 128):
                    nc.tensor.transpose(
                        pst[:, c * 128:(c + 1) * 128],
                        etile[:, c * 128:(c + 1) * 128],
                        ident,
                    )
                nc.vector.tensor_copy(
                    eT[:, :, i * 128:(i + 1) * 128],
                    pst[:].rearrange("p (c q) -> p c q", c=E // 128),
                )

            # ---------------- phase A2: xgb = b_ih + b_hh + e @ W_ih^T ----------
            # computed with M=128 matmuls, spilled to DRAM as fp16
            for m in range(NM):
                for n in range(4):
                    ps = ppool.tile([128, 512], F32, name=f"psa_{m}_{n}",
                                    tag="psb", bufs=2)
                    nc.tensor.matmul(
                        ps, ones, ball_sb[:, n * 512:(n + 1) * 512],
                        start=True, stop=False,
                    )
                    for c in range(E // 128):
                        nc.tensor.matmul(
                            ps,
                            eT[:, c, m * 128:(m + 1) * 128],
                            wih_sb[:, c, n * 512:(n + 1) * 512],
                            start=False, stop=(c == E // 128 - 1),
                        )
                    xgb_sb = vpool.tile([128, 512], F16, name=f"xgb_{m}_{n}",
                                        tag="xgb")
                    nc.vector.tensor_copy(xgb_sb, ps)
                    nc.sync.dma_start(
                        out=xgb_dram[m * 128:(m + 1) * 128, n * 512:(n + 1) * 512],
                        in_=xgb_sb,
                    )

            # ---------------- h0 / c0 projection ----------------
            # hT history: block j = h-state before step j (j=0 is h0); [H(4x128), 33*BP]
            histT = cpool.tile([128, H // 128, (T + 1) * BP], F32)
            hist_bf = cpool.tile([128, H // 128, R], BF16)  # h_1..h_T for phase B

            ps_h0 = ppool.tile([BP, H], F32, tag="gate0")
            ps_c0 = ppool.tile([BP, H], F32, tag="gate1")
            for ps, wsb, bsb in ((ps_h0, whT_sb, bh_sb), (ps_c0, wcT_sb, bc_sb)):
                nc.tensor.matmul(ps, ones[:, :BP], bsb, start=True, stop=False)
                for c, (k0, kn) in enumerate(KCH):
                    nc.tensor.matmul(
                        ps, embT_sb[:kn, c, :], wsb[:kn, c, :],
                        start=False, stop=(c == 2),
                    )
            h0_sb = epool.tile([BP, H], F32)
            nc.scalar.activation(h0_sb, ps_h0, AF.Copy)
            c_cur = spool.tile([BP, H], F32)
            nc.scalar.activation(c_cur, ps_c0, AF.Copy)
            # transpose h0 into histT block 0
            ps_t0 = ppool.tile([128, H // 128 * BP], F32, tag="tp", bufs=2)
            for c in range(H // 128):
                nc.tensor.transpose(
                    ps_t0[:, c * BP:(c + 1) * BP],
                    h0_sb[:, c * 128:(c + 1) * 128],
                    ident[:BP, :BP],
                )
            nc.vector.tensor_copy(
                histT[:, :, 0:BP],
                ps_t0[:].rearrange("p (c q) -> p c q", c=H // 128),
            )

            # ---------------- LSTM over T steps ----------------
            h_last = None
            for t in range(1 if skip_lstm else T):
                xgt = spool.tile([BP, G], F16, name=f"xgt_{t}", tag="xgt", bufs=3)
                nc.sync.dma_start(out=xgt, in_=xgb_dram[t * BP:(t + 1) * BP, :])
                # gates in 4 PSUM banks; recurrent matmuls only.
                # order f,i,g,o so the c-chain can start before o finishes
                gps = [ppool.tile([BP, 512], F32, name=f"gate{n}_{t}", tag=f"gate{n}")
                       for n in range(4)]
                gts = epool.tile([BP, G], F32, name=f"gts_{t}", tag="gts")
                for n in (1, 0, 2, 3):  # f, i, g, o
                    for c in range(H // 128):
                        nc.tensor.matmul(
                            gps[n],
                            histT[:, c, t * BP:(t + 1) * BP],
                            whh_sb[:, c, n * 512:(n + 1) * 512],
                            start=(c == 0), stop=(c == H // 128 - 1),
                        )
                    nc.vector.tensor_add(
                        gts[:, n * 512:(n + 1) * 512], gps[n],
                        xgt[:, n * 512:(n + 1) * 512],
                    )
                if_s = epool.tile([BP, 1024], F32, name=f"if_{t}", tag="if_s")
                nc.scalar.activation(if_s, gts[:, 0:1024], AF.Sigmoid)
                g_t = epool.tile([BP, 512], F32)
                nc.scalar.activation(g_t, gts[:, 1024:1536], AF.Tanh)
                o_s = epool.tile([BP, 512], F32)
                nc.scalar.activation(o_s, gts[:, 1536:2048], AF.Sigmoid)

                fc = epool.tile([BP, 512], F32)
                nc.vector.tensor_mul(fc, if_s[:, 512:1024], c_cur)
                ig = epool.tile([BP, 512], F32)
                nc.vector.tensor_mul(ig, if_s[:, 0:512], g_t)
                c_new = spool.tile([BP, 512], F32, name=f"c_{t}", tag="c_new")
                nc.vector.tensor_add(c_new, fc, ig)
                tc_ = epool.tile([BP, 512], F32)
                nc.scalar.activation(tc_, c_new, AF.Tanh)
                h_new = spool.tile([BP, 512], F32, name=f"h_{t}", tag="h_new")
                nc.vector.tensor_mul(h_new, o_s, tc_)

                # transpose h -> histT block t+1 (+ bf16 copy for phase B)
                ps_t = ppool.tile([128, H // 128 * BP], F32, name=f"pst_{t}",
                                  tag="tp", bufs=2)
                for c in range(H // 128):
                    nc.tensor.transpose(
                        ps_t[:, c * BP:(c + 1) * BP],
                        h_new[:, c * 128:(c + 1) * 128],
                        ident[:BP, :BP],
                    )
                ps_t3 = ps_t[:].rearrange("p (c q) -> p c q", c=H // 128)
                nc.vector.tensor_copy(histT[:, :, (t + 1) * BP:(t + 2) * BP], ps_t3)
                nc.vector.tensor_copy(hist_bf[:, :, t * BP:(t + 1) * BP], ps_t3)

                c_cur = c_new
                h_last = h_new

            if h_last is not None:
                nc.sync.dma_start(out=out_h[:], in_=h_last)
                nc.sync.dma_start(out=out_c[:], in_=c_cur)

            # ---------------- phase B pass 1: z+b tiles (fp16) + exp row-sums ----
            acc = cpool.tile([128, NM, NN], F32)
            for n in range(1 if skip_b1 else NN):
                wt = wpool.tile([128, H // 128, NV], BF16, name=f"wt_{n}", tag="wt")
                nc.sync.dma_start(
                    out=wt,
                    in_=WoT[:, n * NV:(n + 1) * NV].rearrange("(c p) v -> p c v", p=128),
                )
                bt = wpool.tile([128, NV], BF16, name=f"bt_{n}", tag="bt")
                bo_ap = bo[0, n * NV:(n + 1) * NV]
                nc.sync.dma_start(
                    out=bt,
                    in_=bass.AP(tensor=bo_ap.tensor, offset=bo_ap.offset,
                                ap=[[0, 128]] + list(bo_ap.ap)),
                )
                for m in range(NM):
                    ps = ppool.tile([128, NV], F32, name=f"psb1_{n}_{m}",
                                    tag=["psb", "gate0", "gate1", "gate2"][m],
                                    bufs=[2, 1, 1, 1][m])
                    for c in range(H // 128):
                        nc.tensor.matmul(
                            ps,
                            hist_bf[:, c, m * 128:(m + 1) * 128],
                            wt[:, c, :],
                            start=(c == 0), stop=(c == H // 128 - 1),
                        )
                    zb = vpool.tile([128, NV], F16, name=f"zb_{n}_{m}", tag="zb")
                    nc.vector.tensor_add(zb, ps, bt)
                    nc.sync.dma_start(out=zb_dram[m, n // 2, :, (n % 2) * NV:(n % 2 + 1) * NV], in_=zb)
                    esc = vpool.tile([128, NV], F32, name=f"esc_{n}_{m}", tag="ev")
                    nc.scalar.activation(
                        esc, zb, AF.Exp, accum_out=acc[:, m, n:n + 1]
                    )
            lse = cpool.tile([128, NM], F32)
            srow = cpool.tile([128, NM], F32)
            for m in range(NM):
                nc.vector.tensor_reduce(
                    srow[:, m:m + 1], acc[:, m, :], axis=mybir.AxisListType.X,
                    op=ALU.add,
                )
            nc.scalar.activation(lse, srow, AF.Ln)
            nlse = cpool.tile([128, NM], F32)
            nc.vector.tensor_scalar_mul(nlse, lse, -1.0)

            # ---------------- phase B pass 2: out = (z + b) - lse ----------------
            W2 = 2 * NV
            for q in range(1 if skip_b2 else NN // 2):
                for m in range(NM):
                    zbt = vpool.tile([128, W2], F16, name=f"zbt_{q}_{m}", tag="zb")
                    nc.sync.dma_start(out=zbt, in_=zb_dram[m, q])
                    osb = vpool.tile([128, W2], F32, name=f"osb_{q}_{m}", tag="ev")
                    nc.vector.tensor_scalar_add(osb, zbt, nlse[:, m:m + 1])
                    nc.sync.dma_start(
                        out=out_lp[m * 8:(m + 1) * 8, :, q * W2:(q + 1) * W2],
                        in_=osb,
                    )
    nc.compile()
    return nc


_NC_CACHE = {}


def kernel(x_seq, emb, projectEmb, embedding, W_ih, b_ih, W_hh, b_hh,
           W_out, b_out, W_h, b_h, W_c, b_c):
    assert int(projectEmb) == 1
    x_seq = np.asarray(x_seq)
    if "nc" not in _NC_CACHE:
        _NC_CACHE["nc"] = build_nc()
    nc = _NC_CACHE["nc"]

    f32 = np.float32
    WihT = np.ascontiguousarray(np.asarray(W_ih, f32).T)          # [E, G]
    WhhT = np.ascontiguousarray(np.asarray(W_hh, f32).T)          # [H, G]
    ball = (np.asarray(b_ih, f32) + np.asarray(b_hh, f32))[None]  # [1, G]
    WoT = np.ascontiguousarray(np.asarray(W_out, f32).T).astype(ml_dtypes.bfloat16)
    bo = np.asarray(b_out, f32)[None].astype(ml_dtypes.bfloat16)
    WhT = np.ascontiguousarray(np.asarray(W_h, f32).T)            # [DW, H]
    WcT = np.ascontiguousarray(np.asarray(W_c, f32).T)
    bh = np.asarray(b_h, f32)[None]
    bc = np.asarray(b_c, f32)[None]
    embT_full = np.ascontiguousarray(np.asarray(emb, f32).T)      # [DW, B]
    emb_tab = np.ascontiguousarray(np.asarray(embedding, f32))

    in_maps = []
    for r in range(NCORES):
        xs = np.ascontiguousarray(x_seq[:, r * BP:(r + 1) * BP]).reshape(-1)
        idx = np.ascontiguousarray(xs.reshape(NM, 128).T).astype(np.int32)
        in_maps.append({
            "idx": idx,
            "emb_tab": emb_tab,
            "embT": np.ascontiguousarray(embT_full[:, r * BP:(r + 1) * BP]),
            "WhT": WhT, "WcT": WcT, "bh": bh, "bc": bc,
            "WihT": WihT, "WhhT": WhhT, "ball": ball,
            "WoT": WoT, "bo": bo,
        })

    res = run_bass_kernel_spmd(nc, in_maps, core_ids=list(range(NCORES))).results
    logp = np.concatenate([res[r]["out_lp"] for r in range(NCORES)], axis=1)
    h_ = np.concatenate([res[r]["out_h"] for r in range(NCORES)], axis=0)
    c_ = np.concatenate([res[r]["out_c"] for r in range(NCORES)], axis=0)
    return logp, h_, c_
